# revision 6
# baseline (speedup 1.0000x reference)
"""ArgmaxIOU kernel v21 for 8 Trainium2 NeuronCores (shipped).

Best measured 133730 ns, rel err 9.5e-04, tail-after-last-byte 8.65us
(fast device period; slow periods where the DMA fleet drops to
~335-350 GB/s add ~14us).

v21 = v20 + 64 columns shifted from tile 3 into tile 2 (the flat
(e tb) eq view frees tile boundaries from 256-col alignment): the
endgame crunch window loses ~2.7us of DVE work, absorbed by tile 2's
slack.

v18 = v12 + tail/epilogue trims:
 - eq skips class 20 (row 20 of the eq buffers is constant 1.0; the host
   reconstructs row/col 20 from the pixel row/col sums)
 - tile-3 eq as one 384-col op spanning eq slots e2:e3
 - 3-op final fold on the last tile

Data-parallel over batch: core i processes sample i (B=8, C=21, H=W=512).
Raw Bass (explicit engines + semaphores).

v8 = DVE op-count diet (v7 trace showed DMA at 387 GB/s and DVE as the
new bottleneck at 127.7us busy, bloated by ~110 overhead-dominated
narrow-tile ops):
 - tree21 (6 wide tensor_tensor ops, one sem wait) for every tile
   except the last; quarter-tree only on the final 128-col tile where
   the post-last-byte latency needs it.
 - 512-col eq ops that fill both eq ring slots (subs 2k, 2k+1) in one
   instruction.
 - tiles (512, 512, 512, 384, 128); tile 0 emitted as a small 2-class
   load first so the SDMA doorbell rings early.

Host: gather 8 packed [84,84] matrices, fold G, compute mean IoU.
"""

import sys

import numpy as np

for p in ("/opt/trn_rl_repo",):
    if p not in sys.path:
        sys.path.insert(0, p)

from contextlib import ExitStack

from concourse import bass, mybir
from concourse.bass_utils import run_bass_kernel_spmd

B = 8
C = 21
HW = 512 * 512
P = 128
Q = HW // P              # 2048 pixel columns per partition
G = 4
M = G * C                # 84
EQS = 256                # eq sub-slot columns
TBS = EQS // G           # 64 matmul blocks per eq sub-slot
NEQ = 4                  # eq slot ring depth (even: 512-col eq pair-writes)
NSLOT = 2                # data tile ring depth
SLOTW = 576              # slot width (max tile width)
SLOTE = C * SLOTW

W = [512, 512, 576, 320, 128]
OFF = [0, 512, 1024, 1600, 1920]
NT = len(W)
LAST = NT - 1
QTR = [(0, 5), (5, 10), (10, 15), (15, 21)]

F32 = mybir.dt.float32
BF16 = mybir.dt.bfloat16


def build():
    nc = bass.Bass()
    pred = nc.declare_dram_parameter("prediction", [C, HW], F32, isOutput=False)
    targ = nc.declare_dram_parameter("target", [C, HW], F32, isOutput=False)
    out = nc.declare_dram_parameter("out", [M, M], F32, isOutput=True)

    predv = pred[:].rearrange("c (p q) -> p c q", p=P)
    targv = targ[:].rearrange("c (p q) -> p c q", p=P)

    mx = mybir.AluOpType.max
    eqop = mybir.AluOpType.is_equal

    # ---- load bookkeeping: one sem per load ----
    # tile 0 target: 2 loads (classes 0-2 doorbell + 2-21); last tile:
    # class quarters; everything else: one full-tile load.
    def groups_of(k, tid):
        if k <= 1:
            return [(0, 21)]
        if k == LAST:
            return [(0, 16), (16, 21)]
        return [(0, 10), (10, 21)]

    def loads_of(k, tid):
        return len(groups_of(k, tid))

    # emission order (must match the gpsimd program exactly); the shared
    # dm sem hits 16*(i+1) when load ordinal i (and all before it) is done
    emit_order = []
    for k in (0, 1):
        for tid in (0, 1):
            emit_order += [(k, tid, qi) for qi in range(loads_of(k, tid))]
    for k in (2, 3, 4):
        for tid in (0, 1):
            emit_order += [(k, tid, qi) for qi in range(loads_of(k, tid))]
    sidx = {key: i for i, key in enumerate(emit_order)}

    # ---- eq piece bookkeeping (global columns) ----
    pieces_of = {
        0: [(0, 512)],
        1: [(512, 512)],
        2: [(1024, 512), (1536, 64)],
        3: [(1600, 320)],
        4: [(1920, 64), (1984, 64)],
    }

    eqpos = {}
    pos = 0
    for k in range(NT):
        for tid in (0, 1):
            for i in range(len(pieces_of[k])):
                pos += 1
                eqpos[(k, tid, i)] = pos
    n_eq = pos

    eq_done = {k: eqpos[(k, 1, len(pieces_of[k]) - 1)] for k in range(NT)}
    ring_gate = {k: eq_done[k - NSLOT] for k in range(NSLOT, NT)}

    # PE schedule: per sub j, list of (b0, nb, dve_gate)
    NSUB = Q // EQS                      # 8
    pe_sched = {j: [] for j in range(NSUB)}
    for k in range(NT):
        for i, (gcol, glen) in enumerate(pieces_of[k]):
            gate = eqpos[(k, 1, i)]
            j0 = gcol // EQS
            j1 = (gcol + glen - 1) // EQS
            for j in range(j0, j1 + 1):
                lo = max(gcol, j * EQS)
                hi = min(gcol + glen, (j + 1) * EQS)
                pe_sched[j].append(((lo % EQS) // G, (hi - lo) // G, gate))

    with ExitStack() as ctx:
        e_ = ctx.enter_context
        bt = e_(nc.sbuf_tensor("bt", [P, NSLOT * SLOTE], BF16))
        bp = e_(nc.sbuf_tensor("bp", [P, NSLOT * SLOTE], BF16))
        eqt = e_(nc.sbuf_tensor("eqt", [P, NEQ, TBS, C, G], BF16))
        eqp = e_(nc.sbuf_tensor("eqp", [P, NEQ, TBS, C, G], BF16))
        st = e_(nc.sbuf_tensor("st", [P, 10, SLOTW], BF16))
        mo = e_(nc.sbuf_tensor("mo", [P, SLOTW], BF16))
        osb = e_(nc.sbuf_tensor("osb", [M, M], F32))
        conf = e_(nc.psum_tensor("conf", [M, M], F32))
        dms = [e_(nc.semaphore(f"dm{i}")) for i in range(len(emit_order))]
        dve = e_(nc.semaphore("dve"))
        mm = e_(nc.semaphore("mm"))
        fin = e_(nc.semaphore("fin"))
        block = e_(nc.Block())

        srcs = {0: targv, 1: predv}
        pools = {0: bt, 1: bp}

        def tile_view(tid, k):
            s = k % NSLOT
            return pools[tid][:, s * SLOTE:(s + 1) * SLOTE] \
                .rearrange("p (c t) -> p c t", c=C)

        @block.gpsimd
        def _(g):
            def emit(k, tid):
                tv = tile_view(tid, k)
                w = W[k]
                for qi, (clo, chi) in enumerate(groups_of(k, tid)):
                    g.dma_start(
                        out=tv[:, clo:chi, 0:w],
                        in_=srcs[tid][:, clo:chi, OFF[k]:OFF[k] + w],
                    ).then_inc(dms[sidx[(k, tid, qi)]], 16)

            for k in range(NSLOT):
                for tid in (0, 1):
                    emit(k, tid)
            for k in range(NSLOT, NT):
                g.wait_ge(dve, ring_gate[k])
                emit(k, 0)
                emit(k, 1)

        @block.vector
        def _(v):
            def tree21(k, tid):
                # full tile landed: 6 wide ops -> mo[0:w]
                data = tile_view(tid, k)
                w = W[k]
                for qi in range(loads_of(k, tid)):
                    v.wait_ge(dms[sidx[(k, tid, qi)]], 16)
                v.tensor_tensor(st[:, 0:10, 0:w], data[:, 0:10, 0:w],
                                data[:, 10:20, 0:w], mx)
                v.tensor_tensor(st[:, 0:5, 0:w], st[:, 0:5, 0:w], st[:, 5:10, 0:w], mx)
                v.tensor_tensor(st[:, 0:2, 0:w], st[:, 0:2, 0:w], st[:, 2:4, 0:w], mx)
                v.tensor_tensor(st[:, 0:1, 0:w], st[:, 0:1, 0:w], st[:, 1:2, 0:w], mx)
                v.tensor_tensor(st[:, 0:1, 0:w], st[:, 0:1, 0:w], st[:, 4:5, 0:w], mx)
                v.tensor_tensor(mo[:, 0:w].unsqueeze(1), st[:, 0:1, 0:w],
                                data[:, 20:21, 0:w], mx)

            def tree_halves(k, tid):
                # classes 0-9 once the first half-load lands
                data = tile_view(tid, k)
                w = W[k]
                v.wait_ge(dms[sidx[(k, tid, 0)]], 16)
                v.tensor_tensor(st[:, 0:5, 0:w], data[:, 0:5, 0:w], data[:, 5:10, 0:w], mx)
                v.tensor_tensor(st[:, 0:2, 0:w], st[:, 0:2, 0:w], st[:, 2:4, 0:w], mx)
                v.tensor_tensor(st[:, 0:1, 0:w], st[:, 0:1, 0:w], st[:, 1:2, 0:w], mx)
                v.tensor_tensor(st[:, 0:1, 0:w], st[:, 0:1, 0:w], st[:, 4:5, 0:w], mx)
                v.wait_ge(dms[sidx[(k, tid, 1)]], 16)
                v.tensor_tensor(st[:, 1:6, 0:w], data[:, 10:15, 0:w], data[:, 15:20, 0:w], mx)
                v.tensor_tensor(st[:, 1:3, 0:w], st[:, 1:3, 0:w], st[:, 3:5, 0:w], mx)
                v.tensor_tensor(st[:, 1:2, 0:w], st[:, 1:2, 0:w], st[:, 2:3, 0:w], mx)
                v.tensor_tensor(st[:, 1:2, 0:w], st[:, 1:2, 0:w], st[:, 5:6, 0:w], mx)
                v.tensor_tensor(st[:, 1:2, 0:w], st[:, 1:2, 0:w], data[:, 20:21, 0:w], mx)
                v.tensor_tensor(mo[:, 0:w].unsqueeze(1), st[:, 0:1, 0:w],
                                st[:, 1:2, 0:w], mx)

            def tree_tail(k, tid):
                # last tile: classes 0-14 folded while the final 6-class
                # load is in flight; only 4 small ops after the last byte
                data = tile_view(tid, k)
                w = W[k]
                v.wait_ge(dms[sidx[(k, tid, 0)]], 16)
                v.tensor_tensor(st[:, 0:8, 0:w], data[:, 0:8, 0:w], data[:, 8:16, 0:w], mx)
                v.tensor_tensor(st[:, 0:4, 0:w], st[:, 0:4, 0:w], st[:, 4:8, 0:w], mx)
                v.tensor_tensor(st[:, 0:2, 0:w], st[:, 0:2, 0:w], st[:, 2:4, 0:w], mx)
                v.tensor_tensor(st[:, 0:1, 0:w], st[:, 0:1, 0:w], st[:, 1:2, 0:w], mx)
                v.wait_ge(dms[sidx[(k, tid, 1)]], 16)
                v.tensor_tensor(st[:, 1:3, 0:w], data[:, 16:18, 0:w], data[:, 18:20, 0:w], mx)
                v.tensor_tensor(st[:, 1:2, 0:w], st[:, 1:2, 0:w], st[:, 2:3, 0:w], mx)
                v.tensor_tensor(st[:, 1:2, 0:w], st[:, 1:2, 0:w], data[:, 20:21, 0:w], mx)
                v.tensor_tensor(mo[:, 0:w].unsqueeze(1), st[:, 0:1, 0:w],
                                st[:, 1:2, 0:w], mx)

            seen_sub = set()

            def eq_piece(k, tid, i):
                gcol, glen = pieces_of[k][i]
                j0 = gcol // EQS
                j1 = (gcol + glen - 1) // EQS
                need = max((j - (NEQ - 1) for j in range(j0, j1 + 1)
                            if j >= NEQ and j not in seen_sub), default=None)
                for j in range(j0, j1 + 1):
                    seen_sub.add(j)
                if need is not None:
                    v.wait_ge(mm, need)
                data = tile_view(tid, k)
                eqb = eqt if tid == 0 else eqp
                lo = gcol - OFF[k]
                flat = eqb.rearrange("p e tb c g -> p (e tb) c g")
                ep = j0 % NEQ
                b0 = ep * TBS + (gcol % EQS) // G
                nb = glen // G
                outv = flat[:, b0:b0 + nb, 0:20].rearrange("p tb c g -> p c tb g")
                v.tensor_tensor(
                    outv,
                    data[:, 0:20, lo:lo + glen]
                        .rearrange("p c (tb g) -> p c tb g", g=G),
                    mo[:, lo:lo + glen]
                        .rearrange("p (tb g) -> p tb g", g=G)
                        .unsqueeze(1).broadcast_to((P, 20, glen // G, G)),
                    eqop).then_inc(dve, 1)

            ones_done = [False]
            for k in range(NT):
                for tid in (0, 1):
                    if k == LAST:
                        tree_tail(k, tid)
                    elif k <= 1:
                        tree21(k, tid)
                    else:
                        tree_halves(k, tid)
                    if not ones_done[0]:
                        ones_done[0] = True
                        onesrc = mo[:, 0:1].unsqueeze(1).unsqueeze(1) \
                            .broadcast_to((P, NEQ, TBS, G))
                        for eqb_ in (eqt, eqp):
                            v.tensor_tensor(eqb_[:, :, :, 20, :], onesrc,
                                            onesrc, mybir.AluOpType.is_ge)
                    for i in range(len(pieces_of[k])):
                        eq_piece(k, tid, i)

            v.wait_ge(mm, NSUB)
            v.tensor_scalar_add(osb[:], conf[:], 0.0).then_inc(dve, 1)

        @block.tensor
        def _(te):
            first = True
            for j in range(NSUB):
                e = j % NEQ
                for (b0, nb, gate) in pe_sched[j]:
                    te.wait_ge(dve, gate)
                    for tb in range(b0, b0 + nb):
                        inst = te.matmul(
                            conf[:],
                            eqt[:, e, tb].rearrange("p c g -> p (c g)"),
                            eqp[:, e, tb].rearrange("p c g -> p (c g)"),
                            start=first,
                            stop=(j == NSUB - 1
                                  and (b0, nb, gate) == pe_sched[j][-1]
                                  and tb == b0 + nb - 1))
                        first = False
                inst.then_inc(mm, 1)

        @block.sync
        def _(sy):
            sy.wait_ge(dve, n_eq + 1)
            sy.dma_start(out=out[:], in_=osb[:]).then_inc(fin, 16)
            sy.wait_ge(fin, 16)

    return nc


def _score_from_packed(packed):
    """packed: [84, 84] f32 -> per-sample mean IoU (float64)."""
    x = packed.astype(np.float64).reshape(C, G, C, G)
    conf = np.einsum("igjg->ij", x)
    A = conf[0:20, 0:20]
    r = conf[0:20, 20] - A.sum(axis=1)
    c = conf[20, 0:20] - A.sum(axis=0)
    t = conf[20, 20] - conf[0:20, 20].sum() - conf[20, 0:20].sum() + A.sum()
    conf[0:20, 20] = r
    conf[20, 0:20] = c
    conf[20, 20] = t
    TP = np.diag(conf).copy()
    FN = conf.sum(axis=1) - TP
    FP = conf.sum(axis=0) - TP
    valid = TP > 0
    denom = TP + FN + FP
    iou = np.where(valid, TP / np.where(valid, denom, 1.0), 0.0)
    n_valid = max(float(valid.sum()), 1.0)
    return iou.sum() / n_valid


_NC_CACHE = {}


def _get_nc():
    if "nc" not in _NC_CACHE:
        _NC_CACHE["nc"] = build()
    return _NC_CACHE["nc"]


def run(prediction, target, trace=False):
    in_maps = []
    for i in range(B):
        in_maps.append({
            "prediction": np.ascontiguousarray(
                np.asarray(prediction[i], dtype=np.float32).reshape(C, HW)),
            "target": np.ascontiguousarray(
                np.asarray(target[i], dtype=np.float32).reshape(C, HW)),
        })
    res = run_bass_kernel_spmd(_get_nc(), in_maps, core_ids=list(range(B)),
                               trace=trace)
    scores = [_score_from_packed(res.results[i]["out"]) for i in range(B)]
    return np.float32(np.mean(scores)), res


def kernel(prediction, target):
    score, _ = run(prediction, target, trace=False)
    return score
